# revision 23
# baseline (speedup 1.0000x reference)
"""Trainium2 Bass kernel for causal GQA self-attention (8 docs x 1024 tokens,
dim 1024, 16 q heads / 4 kv heads, head_dim 64, RMS-normed+RoPE q/k).

Sharding: data-parallel over docs — core c computes doc c end to end.

Per-core layout strategy (all matmul contractions run over SBUF partitions):
  - x, weights are shipped pre-transposed/chunked from host:
    every [1024, M] "d-major" matrix is stored as [128, 8*M] with the d-chunk
    index folded into the free dim.
  - q/k are produced transposed ([head_dim, token]); v token-major.
  - RMS-norm sums-of-squares via a 0/1 indicator matmul; rsqrt via
    ACT Sqrt + DVE fast reciprocal; per-(head, token) scales broadcast to
    64-row blocks via a tiny K=2 matmul.
  - RoPE via a +-1 permutation matmul (rotate-half) + two elementwise muls.
  - scores^T[k, q] per (head, k-chunk) over q in [128m, 1024); softmax skips
    the max subtraction (scores are bounded by rms-norm: |s| <= 8), exp on
    ACT psum->sbuf bf16; causal triangle masked by a 0/1 multiply (gpsimd).
  - P @ V via v_aug (ones column appended) so softmax denominators fall out
    of the same matmuls; normalization applied to y^T afterwards.
  - final projection straight from y^T (no transposes anywhere).
"""

import os
import sys

sys.path.insert(0, "/opt/trn_rl_repo")

import numpy as np
import ml_dtypes

import concourse.bass as bass
import concourse.bacc as bacc
import concourse.mybir as mybir
import concourse.tile as tile
from concourse import bass_utils
from contextlib import ExitStack

f32 = mybir.dt.float32
f32r = mybir.dt.float32r
bf16 = mybir.dt.bfloat16
BF = ml_dtypes.bfloat16

DIM = 1024
H = 16
HKV = 4
HD = 64
B = 8
S = 1024
NC = 8          # d chunks of 128
QKROWS = DIM + HKV * HD          # 1280
EPS = float(np.finfo(np.float32).eps)
Sqrt = mybir.ActivationFunctionType.Sqrt
Exp = mybir.ActivationFunctionType.Exp

_CACHE = {}


def _build():
    nc = bacc.Bacc("TRN2")
    inp = {}
    for name, shape, dt in [
        ("xr", [128, NC * S], f32r),
        ("wqk", [128, NC * QKROWS], f32r),
        ("wv", [128, NC * 256], f32r),
        ("wp", [128, NC * DIM], f32r),
        ("cosr", [128, S], bf16),
        ("sinr", [128, S], bf16),
        ("rt", [128, 128], bf16),
        ("bsq", [128, 200], bf16),
        ("b2", [2, 128], f32r),
        ("lt", [128, 128], bf16),
        ("gains", [20, 1], f32),
    ]:
        inp[name] = nc.dram_tensor(name, shape, dt, kind="ExternalInput")
    y_out = nc.dram_tensor("y", [S, DIM], f32, kind="ExternalOutput")

    with tile.TileContext(nc) as tc, ExitStack() as top:
        const = top.enter_context(tc.tile_pool(name="const", bufs=1))
        pers = top.enter_context(tc.tile_pool(name="pers", bufs=1))

        sb_cos = const.tile([128, S], bf16, tag="cos", name="sb_cos")
        nc.sync.dma_start(out=sb_cos, in_=inp["cosr"][:])
        sb_sin = const.tile([128, S], bf16, tag="sin", name="sb_sin")
        nc.sync.dma_start(out=sb_sin, in_=inp["sinr"][:])
        sb_rt = const.tile([128, 128], bf16, tag="rt", name="sb_rt")
        nc.sync.dma_start(out=sb_rt, in_=inp["rt"][:])
        sb_bsq = const.tile([128, 200], bf16, tag="bsq", name="sb_bsq")
        nc.sync.dma_start(out=sb_bsq, in_=inp["bsq"][:])
        sb_b2 = const.tile([2, 128], f32r, tag="b2", name="sb_b2")
        nc.sync.dma_start(out=sb_b2, in_=inp["b2"][:])
        sb_lt = const.tile([128, 128], bf16, tag="lt", name="sb_lt")
        nc.sync.dma_start(out=sb_lt, in_=inp["lt"][:])
        sb_gains = const.tile([20, 1], f32, tag="gains", name="sb_gains")
        nc.sync.dma_start(out=sb_gains, in_=inp["gains"][:])

        qf = []   # final scaled+roped qT/kT chunks (bf16), chunks 0-7 q, 8-9 k
        kd = []   # kv head rows duplicated to both partition halves
        vsb = []  # token-major v with ones column per kv head

        # ---------------- stage 1: projections, rms-norm stats, rope -------
        with ExitStack() as s1:
            qr = []
            qrp = s1.enter_context(tc.tile_pool(name="qrp", bufs=1))
            psQ = s1.enter_context(tc.tile_pool(name="psQ", bufs=1, space="PSUM"))
            with ExitStack() as s1a:
                w1 = s1a.enter_context(tc.tile_pool(name="w1", bufs=1))
                sb_x = w1.tile([128, NC * S], f32r, tag="x", name="sb_x")
                sb_wqk = w1.tile([128, NC * QKROWS], f32r, tag="wqk", name="sb_wqk")
                sb_wv = w1.tile([128, NC * 256], f32r, tag="wv", name="sb_wv")
                for kc in range(NC):
                    nc.sync.dma_start(out=sb_x[:, kc * S:(kc + 1) * S],
                                      in_=inp["xr"][:, kc * S:(kc + 1) * S])
                    nc.scalar.dma_start(
                        out=sb_wqk[:, kc * QKROWS:(kc + 1) * QKROWS],
                        in_=inp["wqk"][:, kc * QKROWS:(kc + 1) * QKROWS])
                    nc.scalar.dma_start(out=sb_wv[:, kc * 256:(kc + 1) * 256],
                                        in_=inp["wv"][:, kc * 256:(kc + 1) * 256])

                tmp = s1a.enter_context(tc.tile_pool(name="tmp", bufs=3))
                psA = s1a.enter_context(tc.tile_pool(name="psA", bufs=2, space="PSUM"))
                psR = s1a.enter_context(tc.tile_pool(name="psR", bufs=2, space="PSUM"))
                ps_sq = psQ.tile([20, S], f32, tag="sq", name="ps_sq")

                for c in range(10):
                    ps = psA.tile([128, S], f32, tag="qkv", name=f"ps_qkv{c}")
                    for n in range(2):
                        for kc in range(NC):
                            nc.tensor.matmul(
                                ps[:, n * 512:(n + 1) * 512],
                                lhsT=sb_wqk[:, kc * QKROWS + 128 * c:
                                            kc * QKROWS + 128 * (c + 1)],
                                rhs=sb_x[:, kc * S + n * 512:
                                         kc * S + (n + 1) * 512],
                                start=(kc == 0), stop=(kc == NC - 1))
                    qsb = tmp.tile([128, S], bf16, tag="qs", name=f"qsb{c}")
                    nc.vector.tensor_copy(qsb, ps)
                    q2 = tmp.tile([128, S], bf16, tag="q2", name=f"q2_{c}")
                    nc.vector.tensor_mul(q2, qsb, qsb)
                    for n in range(2):
                        nc.tensor.matmul(
                            ps_sq[:, n * 512:(n + 1) * 512],
                            lhsT=sb_bsq[:, c * 20:(c + 1) * 20],
                            rhs=q2[:, n * 512:(n + 1) * 512],
                            start=(c == 0), stop=(c == 9))
                    qrc = qrp.tile([128, S], bf16, tag=f"qr{c}", name=f"qr{c}")
                    qr.append(qrc)
                    t1 = tmp.tile([128, S], bf16, tag="t1", name=f"t1_{c}")
                    nc.vector.tensor_mul(t1, qsb, sb_cos)
                    for n in range(2):
                        pr = psR.tile([128, 512], f32, tag="rot", name=f"ps_rot{c}_{n}")
                        nc.tensor.matmul(pr, lhsT=sb_rt,
                                         rhs=qsb[:, n * 512:(n + 1) * 512],
                                         start=True, stop=True)
                        t2 = tmp.tile([128, 512], bf16, tag="t2", name=f"t2_{c}_{n}")
                        nc.vector.tensor_mul(t2, pr, sb_sin[:, n * 512:(n + 1) * 512])
                        nc.vector.tensor_add(qrc[:, n * 512:(n + 1) * 512],
                                             t1[:, n * 512:(n + 1) * 512], t2)

                # v: token-major, interleaved [v_g | 1] blocks of 65 cols
                for t in range(NC):
                    psv = psR.tile([128, 256], f32, tag="rot", name=f"ps_v{t}")
                    for kc in range(NC):
                        nc.tensor.matmul(
                            psv,
                            lhsT=sb_x[:, kc * S + 128 * t:
                                      kc * S + 128 * (t + 1)],
                            rhs=sb_wv[:, kc * 256:(kc + 1) * 256],
                            start=(kc == 0), stop=(kc == NC - 1))
                    vt = pers.tile([128, 260], bf16, tag=f"v{t}", name=f"v{t}")
                    vsb.append(vt)
                    vt_g = vt.rearrange("p (g x) -> p g x", x=65)
                    nc.vector.tensor_copy(vt_g[:, :, 0:64],
                                          psv.rearrange("p (g x) -> p g x", x=64))
                    nc.vector.memset(vt_g[:, :, 64:65], 1.0)

            # ---- stage 1b: scales + final q/k ----
            s1b = s1.enter_context(tc.tile_pool(name="s1b", bufs=1))
            psB = s1.enter_context(tc.tile_pool(name="psB", bufs=2, space="PSUM"))
            sb_eps = s1b.tile([20, 1], f32, tag="eps", name="sb_eps")
            nc.vector.memset(sb_eps, EPS)
            sq20 = s1b.tile([20, S], f32, tag="sq20", name="sq20")
            nc.scalar.activation(sq20, ps_sq, Sqrt, scale=1.0 / HD, bias=sb_eps)
            inv20 = s1b.tile([20, S], f32, tag="inv20", name="inv20")
            nc.vector.reciprocal_approx_fast(inv20, sq20)
            sc20 = s1b.tile([20, S], f32r, tag="sc20", name="sc20")
            nc.vector.tensor_scalar_mul(sc20, inv20, sb_gains)
            sq2 = s1b.tile([2, 8 * S], f32r, tag="sq2", name="sq2")
            sk2 = s1b.tile([2, 2 * S], f32r, tag="sk2", name="sk2")
            for j2 in range(2):
                nc.sync.dma_start(out=sq2[j2:j2 + 1, :],
                                  in_=sc20[8 * j2:8 * j2 + 8, :])
                nc.sync.dma_start(out=sk2[j2:j2 + 1, :],
                                  in_=sc20[16 + 2 * j2:18 + 2 * j2, :])
            for c in range(10):
                pb = psB.tile([128, S], f32, tag="bc", name=f"ps_bc{c}")
                rsrc = sq2 if c < 8 else sk2
                roff = c * S if c < 8 else (c - 8) * S
                for n in range(2):
                    nc.tensor.matmul(
                        pb[:, n * 512:(n + 1) * 512],
                        lhsT=sb_b2,
                        rhs=rsrc[:, roff + n * 512:roff + (n + 1) * 512],
                        start=True, stop=True)
                qfc = pers.tile([128, S], bf16, tag=f"qf{c}", name=f"qf{c}")
                qf.append(qfc)
                nc.vector.tensor_mul(qfc, qr[c], pb)
            # duplicate kv-head rows to both halves for base-matched matmuls
            for g in range(4):
                kdg = pers.tile([128, S], bf16, tag=f"kd{g}", name=f"kd{g}")
                kd.append(kdg)
                src = qf[8 + g // 2][(g % 2) * 64:(g % 2) * 64 + 64, :]
                nc.sync.dma_start(out=kdg[0:64, :], in_=src)
                nc.sync.dma_start(out=kdg[64:128, :], in_=src)

        # ---------------- stage 2: attention -------------------------------
        with ExitStack() as s23:
            late = s23.enter_context(tc.tile_pool(name="late", bufs=1))
            yt = [late.tile([128, S], f32r, tag=f"yt{c}", name=f"yt{c}")
                  for c in range(8)]
            s128 = late.tile([128, 128], f32, tag="s128", name="s128")
            s2 = s23.enter_context(ExitStack())
            pP = s2.enter_context(tc.tile_pool(name="pP", bufs=4))
            stg = s2.enter_context(tc.tile_pool(name="stg", bufs=3))
            psS = s2.enter_context(tc.tile_pool(name="psS", bufs=2, space="PSUM"))
            psY = s2.enter_context(tc.tile_pool(name="psY", bufs=4, space="PSUM"))

            # heads processed in pairs: even head at partitions 0-63, odd at
            # 64-127 — interleaved matmuls land on disjoint PE row groups and
            # run concurrently.
            for hp in range(H // 2):
                h0, h1 = 2 * hp, 2 * hp + 1
                cc = hp
                Ppair = {h0: [], h1: []}
                for m in range(NC):
                    w = S - 128 * m
                    pss = {}
                    for h in (h0, h1):
                        pss[h] = psS.tile([128, S], f32, tag="sc",
                                          name=f"ps_sc{h}_{m}")
                    for n0 in range(0, w, 512):
                        nw = min(512, w - n0)
                        for h in (h0, h1):
                            g, b = h // 4, (h % 2) * 64
                            nc.tensor.matmul(
                                pss[h][:, n0:n0 + nw],
                                lhsT=kd[g][b:b + 64, m * 128:(m + 1) * 128],
                                rhs=qf[cc][b:b + 64,
                                           128 * m + n0:128 * m + n0 + nw],
                                start=True, stop=True,
                                skip_group_check=True)
                    for h in (h0, h1):
                        pm = pP.tile([128, w], bf16, tag=f"P{m}", name=f"P{h}_{m}")
                        Ppair[h].append(pm)
                        nc.scalar.activation(pm, pss[h][:, 0:w], Exp)
                        nc.gpsimd.tensor_mul(pm[:, 0:128], pm[:, 0:128], sb_lt)
                for h in (h0, h1):
                    g, b = h // 4, (h % 2) * 64
                    P = Ppair[h]
                    yh = stg.tile([65, S], f32r, tag="yh", name=f"yh{h}")
                    for j in range(2):
                        py = psY.tile([65, 512], f32, tag="y", name=f"ps_y{h}_{j}")
                        for m in range(4 * j + 4):
                            if m <= 4 * j:
                                o0, c0, nw = 0, 512 * j - 128 * m, 512
                            else:
                                o0 = 128 * m - 512 * j
                                c0, nw = 0, 512 - o0
                            nc.tensor.matmul(
                                py[:, o0:o0 + nw],
                                lhsT=vsb[m][:, 65 * g:65 * g + 65],
                                rhs=P[m][:, c0:c0 + nw],
                                start=(m == 0), stop=(m == 4 * j + 3),
                                skip_group_check=True)
                        nc.vector.tensor_copy(yh[:, j * 512:(j + 1) * 512], py)
                    nc.sync.dma_start(out=yt[cc][b:b + 64, :], in_=yh[0:64, :])
                    r0 = 64 * (h % 2) + 8 * (h // 2)
                    nc.sync.dma_start(out=s128[r0:r0 + 8, :],
                                      in_=yh[64:65, :].bitcast(f32))

            s2.close()

            # ---------------- stage 2b + 3: normalization, projection -------
            with ExitStack() as s3:
                # prefetch Wproj while the normalization below runs
                w2 = s3.enter_context(tc.tile_pool(name="w2", bufs=1))
                sb_wp = w2.tile([128, NC * DIM], f32r, tag="wp", name="sb_wp")
                for kc in range(NC):
                    nc.scalar.dma_start(
                        out=sb_wp[:, kc * DIM:(kc + 1) * DIM],
                        in_=inp["wp"][:, kc * DIM:(kc + 1) * DIM])

                psN = s3.enter_context(tc.tile_pool(name="psN", bufs=2,
                                                    space="PSUM"))
                s128r = late.tile([128, 128], f32, tag="s128r", name="s128r")
                nc.vector.reciprocal_approx_fast(s128r, s128)
                s2t = late.tile([2, 8 * S], f32r, tag="s2t", name="s2t")
                for j2 in range(2):
                    nc.sync.dma_start(
                        out=s2t[j2:j2 + 1, :],
                        in_=s128r[64 * j2:64 * j2 + 64, :].bitcast(f32r))
                for cc in range(8):
                    pb = psN.tile([128, S], f32, tag="nb", name=f"ps_nb{cc}")
                    for n in range(2):
                        nc.tensor.matmul(
                            pb[:, n * 512:(n + 1) * 512],
                            lhsT=sb_b2,
                            rhs=s2t[:, cc * S + n * 512:cc * S + (n + 1) * 512],
                            start=True, stop=True)
                    nc.vector.tensor_mul(yt[cc], yt[cc], pb)

                psO = s3.enter_context(tc.tile_pool(name="psO", bufs=4, space="PSUM"))
                osb = w2.tile([128, NC * DIM], f32, tag="osb", name="osb_all")
                for t in range(NC):
                    for n in range(2):
                        po = psO.tile([128, 512], f32, tag="o", name=f"ps_o{t}_{n}")
                        for dc in range(NC):
                            nc.tensor.matmul(
                                po,
                                lhsT=yt[dc][:, t * 128:(t + 1) * 128],
                                rhs=sb_wp[:, dc * DIM + n * 512:
                                          dc * DIM + (n + 1) * 512],
                                start=(dc == 0), stop=(dc == NC - 1))
                        nc.vector.tensor_copy(
                            osb[:, t * DIM + n * 512:t * DIM + (n + 1) * 512], po)
                # ship output in two halves so the first overlaps compute
                yv = y_out.rearrange("(t p) d -> p t d", p=128)
                ov = osb.rearrange("p (t d) -> p t d", d=DIM)
                nc.sync.dma_start(out=yv[:, 0:4, :], in_=ov[:, 0:4, :])
                nc.sync.dma_start(out=yv[:, 4:8, :], in_=ov[:, 4:8, :])
    nc.compile()
    return nc


def _host_prep(x, Wq, Wk, Wv, Wproj, q_gain, q_scale, k_scale,
               rotary_cos, rotary_sin):
    """Shared (per-run) host-side tensors; returns dict name->array plus
    per-doc entries as lists."""
    def chunked(mT, m):
        # [1024, m] d-major -> [128, 8*m] with d-chunk folded into free dim
        return np.ascontiguousarray(
            mT.reshape(NC, 128, m).transpose(1, 0, 2).reshape(128, NC * m))

    wqkT = np.concatenate([Wq, Wk], axis=0).T.astype(np.float32)   # [1024,1280]
    shared = {
        "wqk": chunked(wqkT, QKROWS),
        "wv": chunked(Wv.T.astype(np.float32), 256),
        "wp": chunked(Wproj.T.astype(np.float32), DIM),
    }
    # rotate-half permutation (lhsT = R.T), exact in bf16
    R = np.zeros((128, 128), dtype=np.float32)
    for i in range(128):
        if i % 64 < 32:
            R[i, i + 32] = 1.0
        else:
            R[i, i - 32] = -1.0
    shared["rt"] = R.T.astype(BF).copy()
    # sum-of-squares head indicators
    # scale rows are parity-grouped: q head h -> row (h % 2) * 8 + h // 2,
    # kv head g -> row 16 + 2 * (g % 2) + g // 2
    bsq = np.zeros((128, 200), dtype=np.float32)
    for c in range(10):
        for r in range(128):
            if c < 8:
                h = 2 * c + r // 64
                j = (h % 2) * 8 + h // 2
            else:
                g = 2 * (c - 8) + r // 64
                j = 16 + 2 * (g % 2) + g // 2
            bsq[r, c * 20 + j] = 1.0
    shared["bsq"] = bsq.astype(BF)
    b2 = np.zeros((2, 128), dtype=np.float32)
    b2[0, 0:64] = 1.0
    b2[1, 64:128] = 1.0
    shared["b2"] = b2
    ar = np.arange(128)
    shared["lt"] = (ar[:, None] <= ar[None, :]).astype(BF)
    gains = np.empty((20, 1), dtype=np.float32)
    qg = np.asarray(q_gain, np.float32) * float(q_scale) * (HD ** -0.5)
    for h in range(16):
        gains[(h % 2) * 8 + h // 2, 0] = qg[h]
    gains[16:20, 0] = float(k_scale)
    shared["gains"] = gains

    per_core = []
    x = np.asarray(x, np.float32)
    cos = np.asarray(rotary_cos, np.float32).reshape(B * S, HD // 2)
    sin = np.asarray(rotary_sin, np.float32).reshape(B * S, HD // 2)
    for c in range(B):
        xd = x[c * S:(c + 1) * S]                     # [1024 t, 1024 d]
        xr = np.ascontiguousarray(
            xd.T.reshape(NC, 128, S).transpose(1, 0, 2).reshape(128, NC * S))
        cd = cos[c * S:(c + 1) * S].T                 # [32, 1024]
        sd = sin[c * S:(c + 1) * S].T
        per_core.append({
            "xr": xr,
            "cosr": np.tile(cd, (4, 1)).astype(BF),
            "sinr": np.tile(sd, (4, 1)).astype(BF),
        })
    return shared, per_core


def kernel(x, Wq, Wk, Wv, Wproj, q_gain, q_scale, k_scale,
           rotary_cos, rotary_sin, cu_seqlens=None, max_doc_len=None,
           **_ignored):
    x = np.asarray(x, np.float32)
    assert x.shape == (B * S, DIM), x.shape
    if "nc" not in _CACHE:
        _CACHE["nc"] = _build()
    nc = _CACHE["nc"]
    shared, per_core = _host_prep(
        np.asarray(x, np.float32), np.asarray(Wq, np.float32),
        np.asarray(Wk, np.float32), np.asarray(Wv, np.float32),
        np.asarray(Wproj, np.float32), np.asarray(q_gain, np.float32),
        np.asarray(q_scale, np.float32), np.asarray(k_scale, np.float32),
        np.asarray(rotary_cos, np.float32), np.asarray(rotary_sin, np.float32))
    in_maps = [{**shared, **pc} for pc in per_core]
    res = bass_utils.run_bass_kernel_spmd(
        nc, in_maps, core_ids=list(range(B)),
        trace=bool(int(os.environ.get("KERNEL_TRACE", "0"))))
    _CACHE["last_results"] = res
    out = np.concatenate([res.results[c]["y"] for c in range(B)], axis=0)
    return out.astype(np.float32)
